# revision 21
# baseline (speedup 1.0000x reference)
"""Self-contained Trainium2 kernel for nn_AutoregressiveGroupQuerySelfAttention.

Reference computation (B=2, S=2048, H=2048, 16 heads x 128 dim):
    q = (x @ Wq.T) -> heads; k likewise; v likewise
    q, k get RoPE; scores = (q @ k.T) * sqrt(D)   (faithful-to-source bug)
    causal softmax; ctx = attn @ v; out = ctx @ Wo.T

Sharding over 8 NeuronCores: core c = (b, g) with b = c // 4 (batch),
g = c % 4 (head-group of 4 heads = 512 hidden columns).  Each core computes
its head-group's context and a partial output  ctx_g @ Wo.T[g-rows, :];
the host sums the 4 partials per batch element.

v2 structure (single fused schedule, PE kept continuously busy):
  era 1: q/k/v projections + RoPE, chunked over S (CH=256), with per-kt
         weight DMAs so the first matmuls start as soon as the first
         512KB of weights lands.  v is projected from the same fp32r
         x tiles (no separate bf16 x stream).
  era 2: attention, chunk-outer/head-inner so one head's softmax
         (DVE max + scalar exp) overlaps the next head's score matmuls;
         the output projection for chunk c-1 is interleaved one
         query-tile per head-stage as additional PE filler.
         Softmax normalization is folded into the PE transpose of P by
         using diag(1/rowsum) instead of the identity as the transpose
         multiplicand.
"""
import numpy as np
import ml_dtypes

import concourse.bass as bass
import concourse.mybir as mybir
from concourse import bacc
from concourse.tile import TileContext
from concourse.bass_utils import run_bass_kernel_spmd

F32 = mybir.dt.float32
F32R = mybir.dt.float32r
BF16 = mybir.dt.bfloat16
AX = mybir.AxisListType
ALU = mybir.AluOpType
ACTF = mybir.ActivationFunctionType

B, S, H = 2, 2048, 2048
NUM_HEADS, D = 16, 128
N_CORES = 8
NH = 4                     # heads per core
HG = NH * D                # 512
ROPE_BASE = 10000.0

_NC_CACHE = {}
LAST_RESULTS = None        # BassKernelResults of the most recent run (for profiling)
TRACE = False


def _build(S_=S, H_=H, NH_=NH):
    DD = 128
    HG_ = NH_ * DD
    KT = H_ // 128          # 16 contraction tiles
    SQT = S_ // 128         # 16 seq tiles
    CH1 = 256               # era-1 seq chunk
    NCH1 = S_ // CH1
    CH = 512                # era-2 k chunk
    NCHUNK = S_ // CH

    nc = bacc.Bacc()
    xT = nc.declare_dram_parameter("xT", [H_, S_], F32R, isOutput=False)
    wqT = nc.declare_dram_parameter("wqT", [H_, HG_], F32R, isOutput=False)
    wkT = nc.declare_dram_parameter("wkT", [H_, HG_], F32R, isOutput=False)
    wvT = nc.declare_dram_parameter("wvT", [H_, HG_], F32R, isOutput=False)
    woT = nc.declare_dram_parameter("woT", [HG_, H_], BF16, isOutput=False)
    cosT = nc.declare_dram_parameter("cosT", [128, S_], F32, isOutput=False)
    sinT = nc.declare_dram_parameter("sinT", [128, S_], F32, isOutput=False)
    rT = nc.declare_dram_parameter("rT", [128, 128], F32R, isOutput=False)
    ident = nc.declare_dram_parameter("ident", [128, 128], BF16, isOutput=False)
    mask = nc.declare_dram_parameter("mask", [128, 128], BF16, isOutput=False)
    out = nc.declare_dram_parameter("out", [S_, H_], F32, isOutput=True)

    with TileContext(nc) as tc:
        with (
            tc.tile_pool(name="slabs", bufs=1) as slabp,
            tc.tile_pool(name="stats", bufs=3) as statp,
        ):
            qrope = [slabp.tile([128, S_], F32R, tag=f"qrope{h}", name=f"qrope{h}") for h in range(NH_)]
            krope = [slabp.tile([128, S_], F32R, tag=f"krope{h}", name=f"krope{h}") for h in range(NH_)]
            vslab = slabp.tile([128, SQT * HG_], BF16, tag="vslab")

            # ====== era 1: q/k/v projections + RoPE ======
            with (
                tc.tile_pool(name="w1", bufs=1) as wp1,
                tc.tile_pool(name="xin1", bufs=1) as xp1,
                tc.tile_pool(name="tab", bufs=2) as tabp,
                tc.tile_pool(name="work", bufs=2) as workp,
                tc.tile_pool(name="workt", bufs=1) as worktp,
                tc.tile_pool(name="ps1", bufs=3, space="PSUM") as ps1,
                tc.tile_pool(name="psv", bufs=2, space="PSUM") as psv,
                tc.tile_pool(name="psr", bufs=2, space="PSUM") as psr,
            ):
                rT_sb = wp1.tile([128, 128], F32R, tag="rT")
                nc.sync.dma_start(out=rT_sb[:], in_=rT[:])
                wq_t = [wp1.tile([128, HG_], F32R, tag=f"wq{kt}", name=f"wq{kt}") for kt in range(KT)]
                wk_t = [wp1.tile([128, HG_], F32R, tag=f"wk{kt}", name=f"wk{kt}") for kt in range(KT)]
                wv_t = [wp1.tile([128, HG_], F32R, tag=f"wv{kt}", name=f"wv{kt}") for kt in range(KT)]

                xT3 = xT.rearrange("(kt p) s -> p kt s", p=128)
                for sc in range(NCH1):
                    cs = slice(sc * CH1, (sc + 1) * CH1)
                    cos_t = tabp.tile([128, CH1], F32, tag="cos")
                    nc.sync.dma_start(out=cos_t[:], in_=cosT[:, cs])
                    sin_t = tabp.tile([128, CH1], F32, tag="sin")
                    nc.sync.dma_start(out=sin_t[:], in_=sinT[:, cs])
                    xk = []
                    for kt in range(KT):
                        t = xp1.tile([128, CH1], F32R, tag=f"xb{kt}", name=f"xb{kt}")
                        if sc == 0:
                            # interleave k-weights with x so the k-projection
                            # can start after ~1MB of DMA
                            nc.sync.dma_start(out=wk_t[kt][:], in_=wkT[kt * 128:(kt + 1) * 128, :])
                        nc.sync.dma_start(out=t[:], in_=xT3[:, kt, cs])
                        xk.append(t)
                    if sc == 0:
                        for kt in range(KT):
                            nc.sync.dma_start(out=wq_t[kt][:], in_=wqT[kt * 128:(kt + 1) * 128, :])
                        for kt in range(KT):
                            nc.sync.dma_start(out=wv_t[kt][:], in_=wvT[kt * 128:(kt + 1) * 128, :])
                    pending = None

                    def finish_rope(raw, ropes, h, cos_l, sin_l, cs_l):
                        rotps = psr.tile([128, CH1], F32, tag="rot", name="rotps")
                        nc.tensor.matmul(rotps[:], rT_sb[:], raw[:], start=True, stop=True)
                        t1 = worktp.tile([128, CH1], F32, tag="t1", name="t1")
                        nc.vector.tensor_mul(t1[:], rotps[:], sin_l[:])
                        t2 = worktp.tile([128, CH1], F32, tag="t2", name="t2")
                        nc.vector.tensor_mul(t2[:], raw[:].bitcast(F32), cos_l[:])
                        nc.vector.tensor_add(ropes[h][:, cs_l], t1[:], t2[:])

                    for w_t, ropes in ((wk_t, krope), (wq_t, qrope)):
                        for h in range(NH_):
                            ps = ps1.tile([128, CH1], F32, tag="qk")
                            for kt in range(KT):
                                nc.tensor.matmul(
                                    ps[:],
                                    w_t[kt][:, h * 128:(h + 1) * 128],
                                    xk[kt][:],
                                    start=(kt == 0),
                                    stop=(kt == KT - 1),
                                )
                            raw = workp.tile([128, CH1], F32R, tag="raw")
                            nc.vector.tensor_copy(raw[:], ps[:])
                            if pending is not None:
                                finish_rope(*pending)
                            pending = (raw, ropes, h, cos_t, sin_t, cs)
                    finish_rope(*pending)

                    # v projection for this chunk's 2 seq subtiles
                    for t2i in range(CH1 // 128):
                        t = sc * (CH1 // 128) + t2i
                        vps = psv.tile([128, HG_], F32, tag="vps")
                        for kt in range(KT):
                            nc.tensor.matmul(
                                vps[:],
                                xk[kt][:, t2i * 128:(t2i + 1) * 128],
                                wv_t[kt][:],
                                start=(kt == 0),
                                stop=(kt == KT - 1),
                            )
                        nc.scalar.copy(vslab[:, t * HG_:(t + 1) * HG_], vps[:])

            # ====== era 2: attention + output projection ======
            with (
                tc.tile_pool(name="w2", bufs=1) as wp2,
                tc.tile_pool(name="pslab", bufs=3) as pslabp,
                tc.tile_pool(name="ptpool", bufs=2) as ptp,
                tc.tile_pool(name="ctxpool", bufs=1) as ctxp,
                tc.tile_pool(name="ostage", bufs=2) as ostp,
                tc.tile_pool(name="psbig", bufs=4, space="PSUM") as psbig,
                tc.tile_pool(name="pssmall", bufs=2, space="PSUM") as pssmall,
                tc.tile_pool(name="psctx", bufs=2, space="PSUM") as psctx,
            ):
                ident_sb = wp2.tile([128, 128], BF16, tag="ident")
                nc.sync.dma_start(out=ident_sb[:], in_=ident[:])
                mask_sb = wp2.tile([128, 128], BF16, tag="mask")
                nc.sync.dma_start(out=mask_sb[:], in_=mask[:])
                wo_sb = wp2.tile([128, NH_ * H_], BF16, tag="wo")
                nc.sync.dma_start(
                    out=wo_sb[:].rearrange("p (j ho) -> p j ho", j=NH_),
                    in_=woT.rearrange("(j p) ho -> p j ho", p=128),
                )

                ctxT = [ctxp.tile([128, S_], BF16, tag=f"ctxT{h}", name=f"ctxT{h}") for h in range(NH_)]

                def emit_wo(st):
                    ostg = ostp.tile([128, H_], F32, tag="ostg", name="ostg")
                    for hoc in range(H_ // CH):
                        wops = psbig.tile([128, CH], F32, tag="sc", name="wops")
                        for j in range(NH_):
                            nc.tensor.matmul(
                                wops[:],
                                ctxT[j][:, st * 128:(st + 1) * 128],
                                wo_sb[:, j * H_ + hoc * CH: j * H_ + (hoc + 1) * CH],
                                start=(j == 0),
                                stop=(j == NH_ - 1),
                            )
                        nc.scalar.copy(ostg[:, hoc * CH:(hoc + 1) * CH], wops[:])
                    nc.sync.dma_start(out=out[st * 128:(st + 1) * 128, :], in_=ostg[:])

                for c in range(NCHUNK):
                    for h in range(NH_):
                        if c > 0:
                            emit_wo(4 * (c - 1) + h)

                        ptslab = ptp.tile([128, SQT * CH], BF16, tag="pt", name="pt")
                        pt3 = ptslab[:].rearrange("p (t q) -> p t q", t=SQT)

                        def do_transposes(pbf, sq):
                            off = (sq - 4 * c) * 128
                            for t0 in range(0, sq + 1, 4):
                                n = min(4, sq + 1 - t0)
                                tb = pssmall.tile([128, CH], BF16, tag="tps", name="tps")
                                for i in range(n):
                                    t = t0 + i
                                    nc.tensor.transpose(
                                        tb[:, i * 128:(i + 1) * 128],
                                        pbf[t // 4][:, (t % 4) * 128:(t % 4 + 1) * 128],
                                        ident_sb[:],
                                    )
                                nc.vector.tensor_copy(
                                    pt3[:, t0:t0 + n, off:off + 128],
                                    tb[:].rearrange("p (i q) -> p i q", i=4)[:, :n, :],
                                )

                        rcp4 = statp.tile([128, 4], F32, tag="rcp4")
                        pend_tr = None
                        for j, sq in enumerate(range(4 * c, 4 * c + 4)):
                            nch = c + 1
                            dw = (j + 1) * 128
                            scps_list = []
                            for kc in range(c):
                                scps = psbig.tile([128, CH], F32, tag="sc")
                                nc.tensor.matmul(
                                    scps[:],
                                    qrope[h][:, sq * 128:(sq + 1) * 128],
                                    krope[h][:, kc * CH:(kc + 1) * CH],
                                    start=True,
                                    stop=True,
                                )
                                scps_list.append((scps, CH))
                            dps = psbig.tile([128, CH], F32, tag="sc")
                            if dw > 128:
                                nc.tensor.matmul(
                                    dps[:, :dw - 128],
                                    qrope[h][:, sq * 128:(sq + 1) * 128],
                                    krope[h][:, c * CH: c * CH + dw - 128],
                                    start=True,
                                    stop=True,
                                )
                            # causal mask for the diagonal k-tile via PE:
                            # psum <- I.T @ mask, then accumulate the scores
                            nc.tensor.matmul(
                                dps[:, dw - 128:dw],
                                ident_sb[:],
                                mask_sb[:],
                                start=True,
                                stop=False,
                            )
                            nc.tensor.matmul(
                                dps[:, dw - 128:dw],
                                qrope[h][:, sq * 128:(sq + 1) * 128],
                                krope[h][:, sq * 128:(sq + 1) * 128],
                                start=False,
                                stop=True,
                            )
                            scps_list.append((dps, dw))

                            # row max (one PSUM operand per DVE op)
                            negm = statp.tile([128, 1], F32, tag="negm")
                            if nch == 1:
                                nc.vector.tensor_reduce(
                                    negm[:], dps[:, :dw], axis=AX.X, op=ALU.max, negate=True
                                )
                            else:
                                mx = statp.tile([128, 4], F32, tag="mx")
                                for kc, (scps, cols) in enumerate(scps_list):
                                    nc.vector.tensor_reduce(
                                        mx[:, kc:kc + 1], scps[:, :cols], axis=AX.X, op=ALU.max
                                    )
                                nc.vector.tensor_reduce(
                                    negm[:], mx[:, :nch], axis=AX.X, op=ALU.max, negate=True
                                )

                            # unnormalized P in bf16; row sums accumulate on ACT
                            pbf = [
                                pslabp.tile([128, CH], BF16, tag=f"pbf{kc}", name=f"pbf{kc}")
                                for kc in range(nch)
                            ]
                            ssum = statp.tile([128, 4], F32, tag="ssum")
                            for kc, (scps, cols) in enumerate(scps_list):
                                nc.scalar.activation(
                                    pbf[kc][:, :cols],
                                    scps[:, :cols],
                                    ACTF.Exp,
                                    bias=negm[:],
                                    accum_out=ssum[:, kc:kc + 1],
                                )
                            if nch == 1:
                                nc.vector.reciprocal(rcp4[:, j:j + 1], ssum[:, 0:1])
                            else:
                                rsum = statp.tile([128, 1], F32, tag="rsum")
                                nc.vector.tensor_reduce(
                                    rsum[:], ssum[:, :nch], axis=AX.X, op=ALU.add
                                )
                                nc.vector.reciprocal(rcp4[:, j:j + 1], rsum[:])
                            # normalize P in place (per-partition scalar mul)
                            for kc, (scps, cols) in enumerate(scps_list):
                                nc.vector.tensor_scalar_mul(
                                    pbf[kc][:, :cols], pbf[kc][:, :cols], rcp4[:, j:j + 1]
                                )

                            if pend_tr is not None:
                                do_transposes(*pend_tr)
                            pend_tr = (pbf, sq)
                        do_transposes(*pend_tr)

                        tmax = 4 * c + 4
                        ctxps = psctx.tile([128, CH], F32, tag="ctx")
                        for t in range(tmax):
                            c0 = max(0, (t - 4 * c) * 128)
                            nc.tensor.matmul(
                                ctxps[:, c0:CH],
                                vslab[:, t * HG_ + h * 128: t * HG_ + (h + 1) * 128],
                                ptslab[:, t * CH + c0: t * CH + CH],
                                start=(t == 0),
                                stop=(t == tmax - 1),
                            )
                        # ctx is already normalized; stage to bf16 slab
                        nc.scalar.copy(ctxT[h][:, c * CH:(c + 1) * CH], ctxps[:])

                for h in range(NH_):
                    emit_wo(4 * (NCHUNK - 1) + h)

    nc.compile()
    return nc


def _make_tables(S_, D_=128):
    inv_freq = 1.0 / (ROPE_BASE ** (np.arange(0, D_, 2, dtype=np.float32) / D_))
    pos = np.arange(S_, dtype=np.float32)
    ang = pos[:, None] * inv_freq[None, :]
    ang = np.concatenate([ang, ang], axis=1)
    return (
        np.cos(ang).T.astype(np.float32).copy(),
        np.sin(ang).T.astype(np.float32).copy(),
    )


def _make_rot_T(D_=128):
    R = np.zeros((D_, D_), dtype=np.float32)
    half = D_ // 2
    for d in range(half):
        R[d, d + half] = -1.0
    for d in range(half, D_):
        R[d, d - half] = 1.0
    return R.T.copy()


def _make_mask(mask_val=-1e30):
    m = np.zeros((128, 128), dtype=np.float32)
    m[np.triu_indices(128, k=1)] = mask_val
    return m.astype(ml_dtypes.bfloat16)


def kernel(x, Wq, Wk, Wv, Wo):
    """Full inputs in, full output out. Shards over 8 NeuronCores internally."""
    global LAST_RESULTS
    x = np.ascontiguousarray(np.asarray(x, dtype=np.float32))
    Wq = np.asarray(Wq, dtype=np.float32)
    Wk = np.asarray(Wk, dtype=np.float32)
    Wv = np.asarray(Wv, dtype=np.float32)
    Wo = np.asarray(Wo, dtype=np.float32)

    if "nc" not in _NC_CACHE:
        _NC_CACHE["nc"] = _build()
    nc = _NC_CACHE["nc"]

    scale = np.sqrt(np.float32(D))
    cosT, sinT = _make_tables(S)
    rT = _make_rot_T()
    identb = np.eye(128, dtype=ml_dtypes.bfloat16)
    maskt = _make_mask()

    WqT = Wq.T * scale                    # [H, 16*D], scale folded into q path
    WkT = np.ascontiguousarray(Wk.T)
    WvT = np.ascontiguousarray(Wv.T)
    WoT_bf = Wo.T.astype(ml_dtypes.bfloat16)   # [H(in=ctx), H(out)] rows = ctx hidden

    in_maps = []
    for c in range(N_CORES):
        b, g = divmod(c, NH)
        js = slice(g * HG, (g + 1) * HG)
        xT_b = np.ascontiguousarray(x[b].T)
        in_maps.append({
            "xT": xT_b,
            "wqT": np.ascontiguousarray(WqT[:, js]).astype(np.float32),
            "wkT": np.ascontiguousarray(WkT[:, js]),
            "wvT": np.ascontiguousarray(WvT[:, js]),
            "woT": np.ascontiguousarray(WoT_bf[js, :]),
            "cosT": cosT,
            "sinT": sinT,
            "rT": rT,
            "ident": identb,
            "mask": maskt,
        })

    LAST_RESULTS = run_bass_kernel_spmd(
        nc, in_maps, core_ids=list(range(N_CORES)), trace=TRACE
    )
    res = LAST_RESULTS.results

    out = np.zeros((B, S, H), dtype=np.float32)
    for c in range(N_CORES):
        b = c // NH
        out[b] += res[c]["out"]
    return out


# revision 32
# speedup vs baseline: 1.0010x; 1.0010x over previous
"""Self-contained Trainium2 kernel for nn_AutoregressiveGroupQuerySelfAttention.

Reference computation (B=2, S=2048, H=2048, 16 heads x 128 dim):
    q = (x @ Wq.T) -> heads; k likewise; v likewise
    q, k get RoPE; scores = (q @ k.T) * sqrt(D)   (faithful-to-source bug)
    causal softmax; ctx = attn @ v; out = ctx @ Wo.T

Sharding over 8 NeuronCores: core c = (b, g) with b = c // 4 (batch),
g = c % 4 (head-group of 4 heads = 512 hidden columns).  Each core computes
its head-group's context and a partial output  ctx_g @ Wo.T[g-rows, :];
the host sums the 4 partials per batch element.

v2 structure (single fused schedule, PE kept continuously busy):
  era 1: q/k/v projections + RoPE, chunked over S (CH=256), with per-kt
         weight DMAs so the first matmuls start as soon as the first
         512KB of weights lands.  v is projected from the same fp32r
         x tiles (no separate bf16 x stream).
  era 2: attention, chunk-outer/head-inner so one head's softmax
         (DVE max + scalar exp) overlaps the next head's score matmuls;
         the output projection for chunk c-1 is interleaved one
         query-tile per head-stage as additional PE filler.
         Softmax normalization is folded into the PE transpose of P by
         using diag(1/rowsum) instead of the identity as the transpose
         multiplicand.
"""
import numpy as np
import ml_dtypes

import concourse.bass as bass
import concourse.mybir as mybir
from concourse import bacc
from concourse.tile import TileContext
from concourse.bass_utils import run_bass_kernel_spmd

F32 = mybir.dt.float32
F32R = mybir.dt.float32r
BF16 = mybir.dt.bfloat16
AX = mybir.AxisListType
ALU = mybir.AluOpType
ACTF = mybir.ActivationFunctionType

B, S, H = 2, 2048, 2048
NUM_HEADS, D = 16, 128
N_CORES = 8
NH = 4                     # heads per core
HG = NH * D                # 512
ROPE_BASE = 10000.0

_NC_CACHE = {}
LAST_RESULTS = None        # BassKernelResults of the most recent run (for profiling)
TRACE = False


def _build(S_=S, H_=H, NH_=NH):
    DD = 128
    HG_ = NH_ * DD
    KT = H_ // 128          # 16 contraction tiles
    SQT = S_ // 128         # 16 seq tiles
    CH1 = 512               # era-1 seq chunk
    NCH1 = S_ // CH1
    CH = 512                # era-2 k chunk
    NCHUNK = S_ // CH

    nc = bacc.Bacc()
    xT = nc.declare_dram_parameter("xT", [H_, S_], F32R, isOutput=False)
    xbfT = nc.declare_dram_parameter("xbfT", [H_, S_], BF16, isOutput=False)
    wqT = nc.declare_dram_parameter("wqT", [H_, HG_], F32R, isOutput=False)
    wkT = nc.declare_dram_parameter("wkT", [H_, HG_], F32R, isOutput=False)
    wvT = nc.declare_dram_parameter("wvT", [H_, HG_], BF16, isOutput=False)
    woT = nc.declare_dram_parameter("woT", [HG_, H_], BF16, isOutput=False)
    cosT = nc.declare_dram_parameter("cosT", [128, S_], F32, isOutput=False)
    sinT = nc.declare_dram_parameter("sinT", [128, S_], F32, isOutput=False)
    rT = nc.declare_dram_parameter("rT", [128, 128], F32R, isOutput=False)
    ident = nc.declare_dram_parameter("ident", [128, 128], BF16, isOutput=False)
    identf = nc.declare_dram_parameter("identf", [128, 128], F32, isOutput=False)
    onesr = nc.declare_dram_parameter("onesr", [1, 128], BF16, isOutput=False)
    mask = nc.declare_dram_parameter("mask", [128, 128], BF16, isOutput=False)
    out = nc.declare_dram_parameter("out", [S_, H_], F32, isOutput=True)

    with TileContext(nc) as tc:
        with (
            tc.tile_pool(name="slabs", bufs=1) as slabp,
            tc.tile_pool(name="stats", bufs=3) as statp,
        ):
            qrope = [slabp.tile([128, S_], F32R, tag=f"qrope{h}", name=f"qrope{h}") for h in range(NH_)]
            krope = [slabp.tile([128, S_], F32R, tag=f"krope{h}", name=f"krope{h}") for h in range(NH_)]
            vslab = slabp.tile([128, SQT * HG_], BF16, tag="vslab")

            # ====== era 1: q/k/v projections + RoPE ======
            # Two passes over x (k+v first, then q) so only one 4MB fp32
            # weight set is SBUF-resident at a time, keeping the projection
            # matmuls at N=512 (fp32 LDWEIGHTS amortizes poorly below that).
            with (
                tc.tile_pool(name="xin1", bufs=1) as xp1,
                tc.tile_pool(name="xvin", bufs=2) as xvp,
                tc.tile_pool(name="tab", bufs=2) as tabp,
                tc.tile_pool(name="work", bufs=2) as workp,
                tc.tile_pool(name="workt", bufs=1) as worktp,
                tc.tile_pool(name="rtp", bufs=1) as rtp,
                tc.tile_pool(name="ps1", bufs=3, space="PSUM") as ps1,
                tc.tile_pool(name="psv", bufs=2, space="PSUM") as psv,
                tc.tile_pool(name="psr", bufs=2, space="PSUM") as psr,
            ):
                rT_sb = rtp.tile([128, 128], F32R, tag="rT")
                nc.sync.dma_start(out=rT_sb[:], in_=rT[:])
                xT3 = xT.rearrange("(kt p) s -> p kt s", p=128)
                xbf3 = xbfT.rearrange("(kt p) s -> p kt s", p=128)

                def finish_rope(raw, ropes, h, cos_l, sin_l, cs_l):
                    rotps = psr.tile([128, CH1], F32, tag="rot", name="rotps")
                    nc.tensor.matmul(rotps[:], rT_sb[:], raw[:], start=True, stop=True)
                    t1 = worktp.tile([128, CH1], F32, tag="t1", name="t1")
                    nc.vector.tensor_mul(t1[:], rotps[:], sin_l[:])
                    t2 = worktp.tile([128, CH1], F32, tag="t2", name="t2")
                    nc.vector.tensor_mul(t2[:], raw[:].bitcast(F32), cos_l[:])
                    nc.vector.tensor_add(ropes[h][:, cs_l], t1[:], t2[:])

                for pas in range(2):
                    with tc.tile_pool(name=f"w1_{pas}", bufs=1) as wp1:
                        if pas == 0:
                            w_t = [wp1.tile([128, HG_], F32R, tag=f"wk{kt}", name=f"wk{kt}") for kt in range(KT)]
                            wT_dram, ropes = wkT, krope
                            wv_sb = wp1.tile([128, KT * HG_], BF16, tag="wv")
                        else:
                            w_t = [wp1.tile([128, HG_], F32R, tag=f"wq{kt}", name=f"wq{kt}") for kt in range(KT)]
                            wT_dram, ropes = wqT, qrope
                        for sc in range(NCH1):
                            cs = slice(sc * CH1, (sc + 1) * CH1)
                            cos_t = tabp.tile([128, CH1], F32, tag="cos")
                            nc.sync.dma_start(out=cos_t[:], in_=cosT[:, cs])
                            sin_t = tabp.tile([128, CH1], F32, tag="sin")
                            nc.sync.dma_start(out=sin_t[:], in_=sinT[:, cs])
                            xk = []
                            for kt in range(KT):
                                t = xp1.tile([128, CH1], F32R, tag=f"xb{kt}", name=f"xb{kt}")
                                if sc == 0:
                                    nc.sync.dma_start(
                                        out=w_t[kt][:], in_=wT_dram[kt * 128:(kt + 1) * 128, :]
                                    )
                                nc.sync.dma_start(out=t[:], in_=xT3[:, kt, cs])
                                xk.append(t)
                            if pas == 0 and sc == 0:
                                nc.sync.dma_start(
                                    out=wv_sb[:].rearrange("p (kt j) -> p kt j", kt=KT),
                                    in_=wvT.rearrange("(kt p) j -> p kt j", p=128),
                                )
                            pending = None
                            for h in range(NH_):
                                ps = ps1.tile([128, CH1], F32, tag="qk")
                                for kt in range(KT):
                                    nc.tensor.matmul(
                                        ps[:],
                                        w_t[kt][:, h * 128:(h + 1) * 128],
                                        xk[kt][:],
                                        start=(kt == 0),
                                        stop=(kt == KT - 1),
                                    )
                                raw = workp.tile([128, CH1], F32R, tag="raw")
                                nc.vector.tensor_copy(raw[:], ps[:])
                                if pending is not None:
                                    finish_rope(*pending)
                                pending = (raw, ropes, h, cos_t, sin_t, cs)
                            finish_rope(*pending)

                            if pas == 0:
                                # v projection from a bf16 x stream (FWL-fast
                                # LDWEIGHTS), interleaved into the k pass
                                for t2i in range(CH1 // 128):
                                    t = sc * (CH1 // 128) + t2i
                                    xv = xvp.tile([128, KT * 128], BF16, tag="xv")
                                    nc.sync.dma_start(
                                        out=xv[:].rearrange("p (kt s) -> p kt s", kt=KT),
                                        in_=xbf3[:, :, t * 128:(t + 1) * 128],
                                    )
                                    vps = psv.tile([128, HG_], F32, tag="vps")
                                    for kt in range(KT):
                                        nc.tensor.matmul(
                                            vps[:],
                                            xv[:, kt * 128:(kt + 1) * 128],
                                            wv_sb[:, kt * HG_:(kt + 1) * HG_],
                                            start=(kt == 0),
                                            stop=(kt == KT - 1),
                                        )
                                    nc.scalar.copy(vslab[:, t * HG_:(t + 1) * HG_], vps[:])

            # ====== era 2: attention + output projection ======
            with (
                tc.tile_pool(name="w2", bufs=1) as wp2,
                tc.tile_pool(name="pslab", bufs=3) as pslabp,
                tc.tile_pool(name="ptpool", bufs=2) as ptp,
                tc.tile_pool(name="ctxpool", bufs=1) as ctxp,
                tc.tile_pool(name="ostage", bufs=2) as ostp,
                tc.tile_pool(name="psbig", bufs=4, space="PSUM") as psbig,
                tc.tile_pool(name="pssmall", bufs=1, space="PSUM") as pssmall,
                tc.tile_pool(name="psctx", bufs=2, space="PSUM") as psctx,
            ):
                ident_sb = wp2.tile([128, 128], BF16, tag="ident")
                nc.sync.dma_start(out=ident_sb[:], in_=ident[:])
                identf_sb = wp2.tile([128, 128], F32, tag="identf")
                nc.sync.dma_start(out=identf_sb[:], in_=identf[:])
                ones_sb = wp2.tile([1, 128], BF16, tag="onesr")
                nc.sync.dma_start(out=ones_sb[:], in_=onesr[:])
                mask_sb = wp2.tile([128, 128], BF16, tag="mask")
                nc.sync.dma_start(out=mask_sb[:], in_=mask[:])
                wo_sb = wp2.tile([128, NH_ * H_], BF16, tag="wo")
                nc.sync.dma_start(
                    out=wo_sb[:].rearrange("p (j ho) -> p j ho", j=NH_),
                    in_=woT.rearrange("(j p) ho -> p j ho", p=128),
                )

                ctxT = [ctxp.tile([128, S_], BF16, tag=f"ctxT{h}", name=f"ctxT{h}") for h in range(NH_)]

                def emit_wo(st):
                    ostg = ostp.tile([128, H_], F32, tag="ostg", name="ostg")
                    for hoc in range(H_ // CH):
                        wops = psbig.tile([128, CH], F32, tag="sc", name="wops")
                        for j in range(NH_):
                            nc.tensor.matmul(
                                wops[:],
                                ctxT[j][:, st * 128:(st + 1) * 128],
                                wo_sb[:, j * H_ + hoc * CH: j * H_ + (hoc + 1) * CH],
                                start=(j == 0),
                                stop=(j == NH_ - 1),
                            )
                        nc.scalar.copy(ostg[:, hoc * CH:(hoc + 1) * CH], wops[:])
                    nc.sync.dma_start(out=out[st * 128:(st + 1) * 128, :], in_=ostg[:])

                for c in range(NCHUNK):
                    for h in range(NH_):
                        if c > 0:
                            emit_wo(4 * (c - 1) + h)

                        ptslab = ptp.tile([128, SQT * CH], BF16, tag="pt", name="pt")
                        pt3 = ptslab[:].rearrange("p (t q) -> p t q", t=SQT)

                        def do_transposes(pbf, sq):
                            # DMA xbar transpose: each 128-col block of the
                            # bf16 P chunk lands transposed in the pt slab
                            off = (sq - 4 * c) * 128
                            for t0 in range(0, sq + 1, 4):
                                n = min(4, sq + 1 - t0)
                                nc.sync.dma_start(
                                    out=pt3[:, t0:t0 + n, off:off + 128],
                                    in_=pbf[t0 // 4][:, :n * 128],
                                    transpose=True,
                                )

                        rcp4 = statp.tile([128, 4], F32, tag="rcp4")
                        pend_tr = None
                        for j, sq in enumerate(range(4 * c, 4 * c + 4)):
                            nch = c + 1
                            dw = (j + 1) * 128
                            scps_list = []
                            for kc in range(c):
                                scps = psbig.tile([128, CH], F32, tag="sc")
                                nc.tensor.matmul(
                                    scps[:],
                                    qrope[h][:, sq * 128:(sq + 1) * 128],
                                    krope[h][:, kc * CH:(kc + 1) * CH],
                                    start=True,
                                    stop=True,
                                )
                                scps_list.append((scps, CH))
                            dps = psbig.tile([128, CH], F32, tag="sc")
                            if dw > 128:
                                nc.tensor.matmul(
                                    dps[:, :dw - 128],
                                    qrope[h][:, sq * 128:(sq + 1) * 128],
                                    krope[h][:, c * CH: c * CH + dw - 128],
                                    start=True,
                                    stop=True,
                                )
                            # causal mask for the diagonal k-tile via PE:
                            # psum <- I.T @ mask, then accumulate the scores
                            nc.tensor.matmul(
                                dps[:, dw - 128:dw],
                                ident_sb[:],
                                mask_sb[:],
                                start=True,
                                stop=False,
                            )
                            nc.tensor.matmul(
                                dps[:, dw - 128:dw],
                                qrope[h][:, sq * 128:(sq + 1) * 128],
                                krope[h][:, sq * 128:(sq + 1) * 128],
                                start=False,
                                stop=True,
                            )
                            scps_list.append((dps, dw))

                            # row max (one PSUM operand per DVE op)
                            negm = statp.tile([128, 1], F32, tag="negm")
                            if nch == 1:
                                nc.vector.tensor_reduce(
                                    negm[:], dps[:, :dw], axis=AX.X, op=ALU.max, negate=True
                                )
                            else:
                                mx = statp.tile([128, 4], F32, tag="mx")
                                for kc, (scps, cols) in enumerate(scps_list):
                                    nc.vector.tensor_reduce(
                                        mx[:, kc:kc + 1], scps[:, :cols], axis=AX.X, op=ALU.max
                                    )
                                nc.vector.tensor_reduce(
                                    negm[:], mx[:, :nch], axis=AX.X, op=ALU.max, negate=True
                                )

                            # unnormalized P in bf16; row sums accumulate on ACT
                            pbf = [
                                pslabp.tile([128, CH], BF16, tag=f"pbf{kc}", name=f"pbf{kc}")
                                for kc in range(nch)
                            ]
                            ssum = statp.tile([128, 4], F32, tag="ssum")
                            for kc, (scps, cols) in enumerate(scps_list):
                                nc.scalar.activation(
                                    pbf[kc][:, :cols],
                                    scps[:, :cols],
                                    ACTF.Exp,
                                    bias=negm[:],
                                    accum_out=ssum[:, kc:kc + 1],
                                )
                            if nch == 1:
                                nc.vector.reciprocal(rcp4[:, j:j + 1], ssum[:, 0:1])
                            else:
                                rsum = statp.tile([128, 1], F32, tag="rsum")
                                nc.vector.tensor_reduce(
                                    rsum[:], ssum[:, :nch], axis=AX.X, op=ALU.add
                                )
                                nc.vector.reciprocal(rcp4[:, j:j + 1], rsum[:])

                            if pend_tr is not None:
                                do_transposes(*pend_tr)
                            pend_tr = (pbf, sq)
                        do_transposes(*pend_tr)

                        tmax = 4 * c + 4
                        ctxps = psctx.tile([128, CH], F32, tag="ctx")
                        for t in range(tmax):
                            c0 = max(0, (t - 4 * c) * 128)
                            nc.tensor.matmul(
                                ctxps[:, c0:CH],
                                vslab[:, t * HG_ + h * 128: t * HG_ + (h + 1) * 128],
                                ptslab[:, t * CH + c0: t * CH + CH],
                                start=(t == 0),
                                stop=(t == tmax - 1),
                            )
                        # broadcast 1/rowsum along partitions and normalize ctx
                        rowps = pssmall.tile([1, CH], F32, tag="rcpT")
                        for j in range(4):
                            nc.tensor.transpose(
                                rowps[0:1, j * 128:(j + 1) * 128],
                                rcp4[:, j:j + 1],
                                identf_sb[:],
                            )
                        rrow = statp.tile([1, CH], BF16, tag="rrow")
                        nc.vector.tensor_copy(rrow[:], rowps[:])
                        bcps = pssmall.tile([128, CH], F32, tag="bcps")
                        nc.tensor.matmul(bcps[:], ones_sb[:], rrow[:], start=True, stop=True)
                        bcsb = statp.tile([128, CH], BF16, tag="bcsb")
                        nc.vector.tensor_copy(bcsb[:], bcps[:])
                        nc.vector.tensor_mul(
                            ctxT[h][:, c * CH:(c + 1) * CH], ctxps[:], bcsb[:]
                        )

                for h in range(NH_):
                    emit_wo(4 * (NCHUNK - 1) + h)

    nc.compile()
    return nc


def _make_tables(S_, D_=128):
    inv_freq = 1.0 / (ROPE_BASE ** (np.arange(0, D_, 2, dtype=np.float32) / D_))
    pos = np.arange(S_, dtype=np.float32)
    ang = pos[:, None] * inv_freq[None, :]
    ang = np.concatenate([ang, ang], axis=1)
    return (
        np.cos(ang).T.astype(np.float32).copy(),
        np.sin(ang).T.astype(np.float32).copy(),
    )


def _make_rot_T(D_=128):
    R = np.zeros((D_, D_), dtype=np.float32)
    half = D_ // 2
    for d in range(half):
        R[d, d + half] = -1.0
    for d in range(half, D_):
        R[d, d - half] = 1.0
    return R.T.copy()


def _make_mask(mask_val=-1e30):
    m = np.zeros((128, 128), dtype=np.float32)
    m[np.triu_indices(128, k=1)] = mask_val
    return m.astype(ml_dtypes.bfloat16)


def kernel(x, Wq, Wk, Wv, Wo):
    """Full inputs in, full output out. Shards over 8 NeuronCores internally."""
    global LAST_RESULTS
    x = np.ascontiguousarray(np.asarray(x, dtype=np.float32))
    Wq = np.asarray(Wq, dtype=np.float32)
    Wk = np.asarray(Wk, dtype=np.float32)
    Wv = np.asarray(Wv, dtype=np.float32)
    Wo = np.asarray(Wo, dtype=np.float32)

    if "nc" not in _NC_CACHE:
        _NC_CACHE["nc"] = _build()
    nc = _NC_CACHE["nc"]

    scale = np.sqrt(np.float32(D))
    cosT, sinT = _make_tables(S)
    rT = _make_rot_T()
    identb = np.eye(128, dtype=ml_dtypes.bfloat16)
    identf = np.eye(128, dtype=np.float32)
    onesr = np.ones((1, 128), dtype=ml_dtypes.bfloat16)
    maskt = _make_mask()

    WqT = Wq.T * scale                    # [H, 16*D], scale folded into q path
    WkT = np.ascontiguousarray(Wk.T)
    WvT_bf = Wv.T.astype(ml_dtypes.bfloat16)
    WoT_bf = Wo.T.astype(ml_dtypes.bfloat16)   # [H(in=ctx), H(out)] rows = ctx hidden

    in_maps = []
    for c in range(N_CORES):
        b, g = divmod(c, NH)
        js = slice(g * HG, (g + 1) * HG)
        xT_b = np.ascontiguousarray(x[b].T)
        in_maps.append({
            "xT": xT_b,
            "xbfT": xT_b.astype(ml_dtypes.bfloat16),
            "wqT": np.ascontiguousarray(WqT[:, js]).astype(np.float32),
            "wkT": np.ascontiguousarray(WkT[:, js]),
            "wvT": np.ascontiguousarray(WvT_bf[:, js]),
            "woT": np.ascontiguousarray(WoT_bf[js, :]),
            "cosT": cosT,
            "sinT": sinT,
            "rT": rT,
            "ident": identb,
            "identf": identf,
            "onesr": onesr,
            "mask": maskt,
        })

    LAST_RESULTS = run_bass_kernel_spmd(
        nc, in_maps, core_ids=list(range(N_CORES)), trace=TRACE
    )
    res = LAST_RESULTS.results

    out = np.zeros((B, S, H), dtype=np.float32)
    for c in range(N_CORES):
        b = c // NH
        out[b] += res[c]["out"]
    return out


# revision 37
# speedup vs baseline: 1.1601x; 1.1590x over previous
"""Self-contained Trainium2 kernel for nn_AutoregressiveGroupQuerySelfAttention.

Reference computation (B=2, S=2048, H=2048, 16 heads x 128 dim):
    q = (x @ Wq.T) -> heads; k likewise; v likewise
    q, k get RoPE; scores = (q @ k.T) * sqrt(D)   (faithful-to-source bug)
    causal softmax; ctx = attn @ v; out = ctx @ Wo.T

Sharding over 8 NeuronCores: core c = (b, g) with b = c // 4 (batch),
g = c % 4 (head-group of 4 heads = 512 hidden columns).  Each core computes
its head-group's context and a partial output  ctx_g @ Wo.T[g-rows, :];
the host sums the 4 partials per batch element.

v2 structure (single fused schedule, PE kept continuously busy):
  era 1: q/k/v projections + RoPE, chunked over S (CH=256), with per-kt
         weight DMAs so the first matmuls start as soon as the first
         512KB of weights lands.  v is projected from the same fp32r
         x tiles (no separate bf16 x stream).
  era 2: attention, chunk-outer/head-inner so one head's softmax
         (DVE max + scalar exp) overlaps the next head's score matmuls;
         the output projection for chunk c-1 is interleaved one
         query-tile per head-stage as additional PE filler.
         Softmax normalization is folded into the PE transpose of P by
         using diag(1/rowsum) instead of the identity as the transpose
         multiplicand.
"""
import numpy as np
import ml_dtypes

import concourse.bass as bass
import concourse.mybir as mybir
from concourse import bacc
from concourse.tile import TileContext
from concourse.bass_utils import run_bass_kernel_spmd

F32 = mybir.dt.float32
F32R = mybir.dt.float32r
BF16 = mybir.dt.bfloat16
AX = mybir.AxisListType
ALU = mybir.AluOpType
ACTF = mybir.ActivationFunctionType

B, S, H = 2, 2048, 2048
NUM_HEADS, D = 16, 128
N_CORES = 8
NH = 4                     # heads per core
HG = NH * D                # 512
ROPE_BASE = 10000.0

_NC_CACHE = {}
LAST_RESULTS = None        # BassKernelResults of the most recent run (for profiling)
TRACE = False


def _build(S_=S, H_=H, NH_=NH):
    DD = 128
    HG_ = NH_ * DD
    KT = H_ // 128          # 16 contraction tiles
    SQT = S_ // 128         # 16 seq tiles
    CH1 = 512               # era-1 seq chunk
    NCH1 = S_ // CH1
    CH = 512                # era-2 k chunk
    NCHUNK = S_ // CH

    nc = bacc.Bacc()
    xT = nc.declare_dram_parameter("xT", [H_, S_], F32R, isOutput=False)
    xbfT = nc.declare_dram_parameter("xbfT", [H_, S_], BF16, isOutput=False)
    wqT = nc.declare_dram_parameter("wqT", [H_, HG_], F32R, isOutput=False)
    wkT = nc.declare_dram_parameter("wkT", [H_, HG_], F32R, isOutput=False)
    wvT = nc.declare_dram_parameter("wvT", [H_, HG_], BF16, isOutput=False)
    woT = nc.declare_dram_parameter("woT", [HG_, H_], BF16, isOutput=False)
    cosT = nc.declare_dram_parameter("cosT", [128, S_], F32, isOutput=False)
    sinT = nc.declare_dram_parameter("sinT", [128, S_], F32, isOutput=False)
    rT = nc.declare_dram_parameter("rT", [128, 128], F32R, isOutput=False)
    ident = nc.declare_dram_parameter("ident", [128, 128], BF16, isOutput=False)
    identf = nc.declare_dram_parameter("identf", [128, 128], F32, isOutput=False)
    onesr = nc.declare_dram_parameter("onesr", [1, 128], BF16, isOutput=False)
    mask = nc.declare_dram_parameter("mask", [128, 128], BF16, isOutput=False)
    out = nc.declare_dram_parameter("out", [S_, H_], F32, isOutput=True)

    with TileContext(nc) as tc:
        with (
            tc.tile_pool(name="slabs", bufs=1) as slabp,
            tc.tile_pool(name="stats", bufs=3) as statp,
        ):
            qrope = [slabp.tile([128, S_], F32R, tag=f"qrope{h}", name=f"qrope{h}") for h in range(NH_)]
            krope = [slabp.tile([128, S_], F32R, tag=f"krope{h}", name=f"krope{h}") for h in range(NH_)]
            vslab = slabp.tile([128, SQT * HG_], BF16, tag="vslab")

            # ====== era 1: q/k/v projections + RoPE ======
            # Two passes over x (k+v first, then q) so only one 4MB fp32
            # weight set is SBUF-resident at a time, keeping the projection
            # matmuls at N=512 (fp32 LDWEIGHTS amortizes poorly below that).
            with (
                tc.tile_pool(name="xin1", bufs=1) as xp1,
                tc.tile_pool(name="xvin", bufs=1) as xvp,
                tc.tile_pool(name="tab", bufs=2) as tabp,
                tc.tile_pool(name="work", bufs=2) as workp,
                tc.tile_pool(name="workt", bufs=1) as worktp,
                tc.tile_pool(name="rtp", bufs=1) as rtp,
                tc.tile_pool(name="ps1", bufs=3, space="PSUM") as ps1,
                tc.tile_pool(name="psv", bufs=2, space="PSUM") as psv,
                tc.tile_pool(name="psr", bufs=2, space="PSUM") as psr,
            ):
                rT_sb = rtp.tile([128, 128], F32R, tag="rT")
                nc.sync.dma_start(out=rT_sb[:], in_=rT[:])
                xT3 = xT.rearrange("(kt p) s -> p kt s", p=128)
                xbf3 = xbfT.rearrange("(kt p) s -> p kt s", p=128)

                def finish_rope(raw, ropes, h, cos_l, sin_l, cs_l):
                    rotps = psr.tile([128, CH1], F32, tag="rot", name="rotps")
                    nc.tensor.matmul(rotps[:], rT_sb[:], raw[:], start=True, stop=True)
                    t1 = worktp.tile([128, CH1], F32, tag="t1", name="t1")
                    nc.vector.tensor_mul(t1[:], rotps[:], sin_l[:])
                    t2 = worktp.tile([128, CH1], F32, tag="t2", name="t2")
                    nc.vector.tensor_mul(t2[:], raw[:].bitcast(F32), cos_l[:])
                    nc.vector.tensor_add(ropes[h][:, cs_l], t1[:], t2[:])

                for pas in range(2):
                    with tc.tile_pool(name=f"w1_{pas}", bufs=1) as wp1:
                        if pas == 0:
                            w_t = [wp1.tile([128, HG_], F32R, tag=f"wk{kt}", name=f"wk{kt}") for kt in range(KT)]
                            wT_dram, ropes = wkT, krope
                            wv_sb = wp1.tile([128, KT * HG_], BF16, tag="wv")
                        else:
                            w_t = [wp1.tile([128, HG_], F32R, tag=f"wq{kt}", name=f"wq{kt}") for kt in range(KT)]
                            wT_dram, ropes = wqT, qrope
                        for sc in range(NCH1):
                            cs = slice(sc * CH1, (sc + 1) * CH1)
                            cos_t = tabp.tile([128, CH1], F32, tag="cos")
                            nc.sync.dma_start(out=cos_t[:], in_=cosT[:, cs])
                            sin_t = tabp.tile([128, CH1], F32, tag="sin")
                            nc.sync.dma_start(out=sin_t[:], in_=sinT[:, cs])
                            if sc == 0:
                                for kt in range(KT):
                                    nc.sync.dma_start(
                                        out=w_t[kt][:], in_=wT_dram[kt * 128:(kt + 1) * 128, :]
                                    )
                            xk = xp1.tile([128, KT * CH1], F32R, tag="xk")
                            nc.sync.dma_start(
                                out=xk[:].rearrange("p (kt s) -> p kt s", kt=KT),
                                in_=xT3[:, :, cs],
                            )
                            if pas == 0:
                                xv = xvp.tile([128, KT * CH1], BF16, tag="xv")
                                nc.sync.dma_start(
                                    out=xv[:].rearrange("p (kt s) -> p kt s", kt=KT),
                                    in_=xbf3[:, :, cs],
                                )
                            if pas == 0 and sc == 0:
                                nc.sync.dma_start(
                                    out=wv_sb[:].rearrange("p (kt j) -> p kt j", kt=KT),
                                    in_=wvT.rearrange("(kt p) j -> p kt j", p=128),
                                )
                            pending = None
                            for h in range(NH_):
                                ps = ps1.tile([128, CH1], F32, tag="qk")
                                for kt in range(KT):
                                    nc.tensor.matmul(
                                        ps[:],
                                        w_t[kt][:, h * 128:(h + 1) * 128],
                                        xk[:, kt * CH1:(kt + 1) * CH1],
                                        start=(kt == 0),
                                        stop=(kt == KT - 1),
                                    )
                                raw = workp.tile([128, CH1], F32R, tag="raw")
                                nc.vector.tensor_copy(raw[:], ps[:])
                                if pending is not None:
                                    finish_rope(*pending)
                                pending = (raw, ropes, h, cos_t, sin_t, cs)
                            finish_rope(*pending)

                            if pas == 0:
                                # v projection from a bf16 x stream (FWL-fast
                                # LDWEIGHTS), interleaved into the k pass
                                for t2i in range(CH1 // 128):
                                    t = sc * (CH1 // 128) + t2i
                                    vps = psv.tile([128, HG_], F32, tag="vps")
                                    for kt in range(KT):
                                        nc.tensor.matmul(
                                            vps[:],
                                            xv[:, kt * CH1 + t2i * 128: kt * CH1 + (t2i + 1) * 128],
                                            wv_sb[:, kt * HG_:(kt + 1) * HG_],
                                            start=(kt == 0),
                                            stop=(kt == KT - 1),
                                        )
                                    nc.scalar.copy(vslab[:, t * HG_:(t + 1) * HG_], vps[:])

            # ====== era 2: attention + output projection ======
            with (
                tc.tile_pool(name="w2", bufs=1) as wp2,
                tc.tile_pool(name="pslab", bufs=3) as pslabp,
                tc.tile_pool(name="ptpool", bufs=2) as ptp,
                tc.tile_pool(name="ctxpool", bufs=1) as ctxp,
                tc.tile_pool(name="ostage", bufs=2) as ostp,
                tc.tile_pool(name="psbig", bufs=4, space="PSUM") as psbig,
                tc.tile_pool(name="pssmall", bufs=1, space="PSUM") as pssmall,
                tc.tile_pool(name="psctx", bufs=2, space="PSUM") as psctx,
            ):
                ident_sb = wp2.tile([128, 128], BF16, tag="ident")
                nc.sync.dma_start(out=ident_sb[:], in_=ident[:])
                identf_sb = wp2.tile([128, 128], F32, tag="identf")
                nc.sync.dma_start(out=identf_sb[:], in_=identf[:])
                ones_sb = wp2.tile([1, 128], BF16, tag="onesr")
                nc.sync.dma_start(out=ones_sb[:], in_=onesr[:])
                mask_sb = wp2.tile([128, 128], BF16, tag="mask")
                nc.sync.dma_start(out=mask_sb[:], in_=mask[:])
                wo_sb = wp2.tile([128, NH_ * H_], BF16, tag="wo")
                nc.sync.dma_start(
                    out=wo_sb[:].rearrange("p (j ho) -> p j ho", j=NH_),
                    in_=woT.rearrange("(j p) ho -> p j ho", p=128),
                )

                ctxT = [ctxp.tile([128, S_], BF16, tag=f"ctxT{h}", name=f"ctxT{h}") for h in range(NH_)]

                def emit_wo(st):
                    ostg = ostp.tile([128, H_], F32, tag="ostg", name="ostg")
                    for hoc in range(H_ // CH):
                        wops = psbig.tile([128, CH], F32, tag="sc", name="wops")
                        for j in range(NH_):
                            nc.tensor.matmul(
                                wops[:],
                                ctxT[j][:, st * 128:(st + 1) * 128],
                                wo_sb[:, j * H_ + hoc * CH: j * H_ + (hoc + 1) * CH],
                                start=(j == 0),
                                stop=(j == NH_ - 1),
                            )
                        if hoc % 2 == 0:
                            nc.scalar.copy(ostg[:, hoc * CH:(hoc + 1) * CH], wops[:])
                        else:
                            nc.vector.tensor_copy(ostg[:, hoc * CH:(hoc + 1) * CH], wops[:])
                    nc.sync.dma_start(out=out[st * 128:(st + 1) * 128, :], in_=ostg[:])

                for c in range(NCHUNK):
                    for h in range(NH_):
                        if c > 0:
                            emit_wo(4 * (c - 1) + h)

                        ptslab = ptp.tile([128, SQT * CH], BF16, tag="pt", name="pt")
                        pt3 = ptslab[:].rearrange("p (t q) -> p t q", t=SQT)

                        def do_transposes(pbf, sq):
                            # DMA xbar transpose: each 128-col block of the
                            # bf16 P slab lands transposed in the pt slab
                            off = (sq - 4 * c) * 128
                            nc.sync.dma_start(
                                out=pt3[:, 0:sq + 1, off:off + 128],
                                in_=pbf[:, :(sq + 1) * 128],
                                transpose=True,
                            )

                        rcp4 = statp.tile([128, 4], F32, tag="rcp4")
                        pend_tr = None
                        for j, sq in enumerate(range(4 * c, 4 * c + 4)):
                            nch = c + 1
                            dw = (j + 1) * 128
                            scps_list = []
                            for kc in range(c):
                                scps = psbig.tile([128, CH], F32, tag="sc")
                                nc.tensor.matmul(
                                    scps[:],
                                    qrope[h][:, sq * 128:(sq + 1) * 128],
                                    krope[h][:, kc * CH:(kc + 1) * CH],
                                    start=True,
                                    stop=True,
                                )
                                scps_list.append((scps, CH))
                            dps = psbig.tile([128, CH], F32, tag="sc")
                            if dw > 128:
                                nc.tensor.matmul(
                                    dps[:, :dw - 128],
                                    qrope[h][:, sq * 128:(sq + 1) * 128],
                                    krope[h][:, c * CH: c * CH + dw - 128],
                                    start=True,
                                    stop=True,
                                )
                            # causal mask for the diagonal k-tile via PE:
                            # psum <- I.T @ mask, then accumulate the scores
                            nc.tensor.matmul(
                                dps[:, dw - 128:dw],
                                ident_sb[:],
                                mask_sb[:],
                                start=True,
                                stop=False,
                            )
                            nc.tensor.matmul(
                                dps[:, dw - 128:dw],
                                qrope[h][:, sq * 128:(sq + 1) * 128],
                                krope[h][:, sq * 128:(sq + 1) * 128],
                                start=False,
                                stop=True,
                            )
                            scps_list.append((dps, dw))

                            # row max (one PSUM operand per DVE op)
                            negm = statp.tile([128, 1], F32, tag="negm")
                            if nch == 1:
                                nc.vector.tensor_reduce(
                                    negm[:], dps[:, :dw], axis=AX.X, op=ALU.max, negate=True
                                )
                            else:
                                mx = statp.tile([128, 4], F32, tag="mx")
                                for kc, (scps, cols) in enumerate(scps_list):
                                    nc.vector.tensor_reduce(
                                        mx[:, kc:kc + 1], scps[:, :cols], axis=AX.X, op=ALU.max
                                    )
                                nc.vector.tensor_reduce(
                                    negm[:], mx[:, :nch], axis=AX.X, op=ALU.max, negate=True
                                )

                            # unnormalized P in bf16; row sums accumulate on ACT
                            pbf = pslabp.tile([128, S_], BF16, tag="pbf", name="pbf")
                            ssum = statp.tile([128, 4], F32, tag="ssum")
                            for kc, (scps, cols) in enumerate(scps_list):
                                nc.scalar.activation(
                                    pbf[:, kc * CH: kc * CH + cols],
                                    scps[:, :cols],
                                    ACTF.Exp,
                                    bias=negm[:],
                                    accum_out=ssum[:, kc:kc + 1],
                                )
                            if nch == 1:
                                nc.vector.reciprocal(rcp4[:, j:j + 1], ssum[:, 0:1])
                            else:
                                rsum = statp.tile([128, 1], F32, tag="rsum")
                                nc.vector.tensor_reduce(
                                    rsum[:], ssum[:, :nch], axis=AX.X, op=ALU.add
                                )
                                nc.vector.reciprocal(rcp4[:, j:j + 1], rsum[:])

                            if pend_tr is not None:
                                do_transposes(*pend_tr)
                            pend_tr = (pbf, sq)
                        do_transposes(*pend_tr)

                        tmax = 4 * c + 4
                        ctxps = psctx.tile([128, CH], F32, tag="ctx")
                        for t in range(tmax):
                            c0 = max(0, (t - 4 * c) * 128)
                            nc.tensor.matmul(
                                ctxps[:, c0:CH],
                                vslab[:, t * HG_ + h * 128: t * HG_ + (h + 1) * 128],
                                ptslab[:, t * CH + c0: t * CH + CH],
                                start=(t == 0),
                                stop=(t == tmax - 1),
                            )
                        # broadcast 1/rowsum along partitions and normalize ctx
                        rowps = pssmall.tile([1, CH], F32, tag="rcpT")
                        for j in range(4):
                            nc.tensor.transpose(
                                rowps[0:1, j * 128:(j + 1) * 128],
                                rcp4[:, j:j + 1],
                                identf_sb[:],
                            )
                        rrow = statp.tile([1, CH], BF16, tag="rrow")
                        nc.vector.tensor_copy(rrow[:], rowps[:])
                        bcps = pssmall.tile([128, CH], F32, tag="bcps")
                        nc.tensor.matmul(bcps[:], ones_sb[:], rrow[:], start=True, stop=True)
                        bcsb = statp.tile([128, CH], BF16, tag="bcsb")
                        nc.vector.tensor_copy(bcsb[:], bcps[:])
                        nc.vector.tensor_mul(
                            ctxT[h][:, c * CH:(c + 1) * CH], ctxps[:], bcsb[:]
                        )

                for h in range(NH_):
                    emit_wo(4 * (NCHUNK - 1) + h)

    nc.compile()
    return nc


def _make_tables(S_, D_=128):
    inv_freq = 1.0 / (ROPE_BASE ** (np.arange(0, D_, 2, dtype=np.float32) / D_))
    pos = np.arange(S_, dtype=np.float32)
    ang = pos[:, None] * inv_freq[None, :]
    ang = np.concatenate([ang, ang], axis=1)
    return (
        np.cos(ang).T.astype(np.float32).copy(),
        np.sin(ang).T.astype(np.float32).copy(),
    )


def _make_rot_T(D_=128):
    R = np.zeros((D_, D_), dtype=np.float32)
    half = D_ // 2
    for d in range(half):
        R[d, d + half] = -1.0
    for d in range(half, D_):
        R[d, d - half] = 1.0
    return R.T.copy()


def _make_mask(mask_val=-1e30):
    m = np.zeros((128, 128), dtype=np.float32)
    m[np.triu_indices(128, k=1)] = mask_val
    return m.astype(ml_dtypes.bfloat16)


def kernel(x, Wq, Wk, Wv, Wo):
    """Full inputs in, full output out. Shards over 8 NeuronCores internally."""
    global LAST_RESULTS
    x = np.ascontiguousarray(np.asarray(x, dtype=np.float32))
    Wq = np.asarray(Wq, dtype=np.float32)
    Wk = np.asarray(Wk, dtype=np.float32)
    Wv = np.asarray(Wv, dtype=np.float32)
    Wo = np.asarray(Wo, dtype=np.float32)

    if "nc" not in _NC_CACHE:
        _NC_CACHE["nc"] = _build()
    nc = _NC_CACHE["nc"]

    scale = np.sqrt(np.float32(D))
    cosT, sinT = _make_tables(S)
    rT = _make_rot_T()
    identb = np.eye(128, dtype=ml_dtypes.bfloat16)
    identf = np.eye(128, dtype=np.float32)
    onesr = np.ones((1, 128), dtype=ml_dtypes.bfloat16)
    maskt = _make_mask()

    WqT = Wq.T * scale                    # [H, 16*D], scale folded into q path
    WkT = np.ascontiguousarray(Wk.T)
    WvT_bf = Wv.T.astype(ml_dtypes.bfloat16)
    WoT_bf = Wo.T.astype(ml_dtypes.bfloat16)   # [H(in=ctx), H(out)] rows = ctx hidden

    in_maps = []
    for c in range(N_CORES):
        b, g = divmod(c, NH)
        js = slice(g * HG, (g + 1) * HG)
        xT_b = np.ascontiguousarray(x[b].T)
        in_maps.append({
            "xT": xT_b,
            "xbfT": xT_b.astype(ml_dtypes.bfloat16),
            "wqT": np.ascontiguousarray(WqT[:, js]).astype(np.float32),
            "wkT": np.ascontiguousarray(WkT[:, js]),
            "wvT": np.ascontiguousarray(WvT_bf[:, js]),
            "woT": np.ascontiguousarray(WoT_bf[js, :]),
            "cosT": cosT,
            "sinT": sinT,
            "rT": rT,
            "ident": identb,
            "identf": identf,
            "onesr": onesr,
            "mask": maskt,
        })

    LAST_RESULTS = run_bass_kernel_spmd(
        nc, in_maps, core_ids=list(range(N_CORES)), trace=TRACE
    )
    res = LAST_RESULTS.results

    out = np.zeros((B, S, H), dtype=np.float32)
    for c in range(N_CORES):
        b = c // NH
        out[b] += res[c]["out"]
    return out


# revision 47
# speedup vs baseline: 1.2101x; 1.0431x over previous
"""Self-contained Trainium2 kernel for nn_AutoregressiveGroupQuerySelfAttention.

Reference computation (B=2, S=2048, H=2048, 16 heads x 128 dim):
    q = (x @ Wq.T) -> heads; k likewise; v likewise
    q, k get RoPE; scores = (q @ k.T) * sqrt(D)   (faithful-to-source bug)
    causal softmax; ctx = attn @ v; out = ctx @ Wo.T

Sharding over 8 NeuronCores: core c = (b, g) with b = c // 4 (batch),
g = c % 4 (head-group of 4 heads = 512 hidden columns).  Each core computes
its head-group's context and a partial output  ctx_g @ Wo.T[g-rows, :];
the host sums the 4 partials per batch element.

v2 structure (single fused schedule, PE kept continuously busy):
  era 1: q/k/v projections + RoPE, chunked over S (CH=256), with per-kt
         weight DMAs so the first matmuls start as soon as the first
         512KB of weights lands.  v is projected from the same fp32r
         x tiles (no separate bf16 x stream).
  era 2: attention, chunk-outer/head-inner so one head's softmax
         (DVE max + scalar exp) overlaps the next head's score matmuls;
         the output projection for chunk c-1 is interleaved one
         query-tile per head-stage as additional PE filler.
         Softmax normalization is folded into the PE transpose of P by
         using diag(1/rowsum) instead of the identity as the transpose
         multiplicand.
"""
import numpy as np
import ml_dtypes

import concourse.bass as bass
import concourse.mybir as mybir
from concourse import bacc
from concourse.tile import TileContext
from concourse.bass_utils import run_bass_kernel_spmd

F32 = mybir.dt.float32
F32R = mybir.dt.float32r
BF16 = mybir.dt.bfloat16
AX = mybir.AxisListType
ALU = mybir.AluOpType
ACTF = mybir.ActivationFunctionType

B, S, H = 2, 2048, 2048
NUM_HEADS, D = 16, 128
N_CORES = 8
NH = 4                     # heads per core
HG = NH * D                # 512
ROPE_BASE = 10000.0

_NC_CACHE = {}
LAST_RESULTS = None        # BassKernelResults of the most recent run (for profiling)
TRACE = False


def _build(S_=S, H_=H, NH_=NH):
    DD = 128
    HG_ = NH_ * DD
    KT = H_ // 128          # 16 contraction tiles
    SQT = S_ // 128         # 16 seq tiles
    CH1 = 512               # era-1 seq chunk
    NCH1 = S_ // CH1
    CH = 512                # era-2 k chunk
    NCHUNK = S_ // CH

    nc = bacc.Bacc()
    xT = nc.declare_dram_parameter("xT", [H_, S_], F32R, isOutput=False)
    xbfT = nc.declare_dram_parameter("xbfT", [H_, S_], BF16, isOutput=False)
    wqT = nc.declare_dram_parameter("wqT", [H_, HG_], F32R, isOutput=False)
    wkT = nc.declare_dram_parameter("wkT", [H_, HG_], F32R, isOutput=False)
    wvT = nc.declare_dram_parameter("wvT", [H_, HG_], BF16, isOutput=False)
    woT = nc.declare_dram_parameter("woT", [HG_, H_], BF16, isOutput=False)
    cosT = nc.declare_dram_parameter("cosT", [128, S_], F32, isOutput=False)
    sinT = nc.declare_dram_parameter("sinT", [128, S_], F32, isOutput=False)
    rT = nc.declare_dram_parameter("rT", [128, 128], F32R, isOutput=False)
    ident = nc.declare_dram_parameter("ident", [128, 128], BF16, isOutput=False)
    identf = nc.declare_dram_parameter("identf", [128, 128], F32, isOutput=False)
    onesr = nc.declare_dram_parameter("onesr", [1, 128], BF16, isOutput=False)
    mask = nc.declare_dram_parameter("mask", [128, 128], F32, isOutput=False)
    out = nc.declare_dram_parameter("out", [S_, H_], F32, isOutput=True)

    with TileContext(nc) as tc:
        with (
            tc.tile_pool(name="slabs", bufs=1) as slabp,
            tc.tile_pool(name="stats", bufs=3) as statp,
        ):
            qrope = [slabp.tile([128, S_], F32R, tag=f"qrope{h}", name=f"qrope{h}") for h in range(NH_)]
            krope = [slabp.tile([128, S_], F32R, tag=f"krope{h}", name=f"krope{h}") for h in range(NH_)]
            vslab = slabp.tile([128, SQT * HG_], BF16, tag="vslab")

            # ====== era 1: q/k/v projections + RoPE ======
            # Two passes over x (k+v first, then q) so only one 4MB fp32
            # weight set is SBUF-resident at a time, keeping the projection
            # matmuls at N=512 (fp32 LDWEIGHTS amortizes poorly below that).
            with (
                tc.tile_pool(name="xin1", bufs=1) as xp1,
                tc.tile_pool(name="xvin", bufs=1) as xvp,
                tc.tile_pool(name="tab", bufs=2) as tabp,
                tc.tile_pool(name="work", bufs=2) as workp,
                tc.tile_pool(name="workt", bufs=1) as worktp,
                tc.tile_pool(name="rtp", bufs=1) as rtp,
                tc.tile_pool(name="ps1", bufs=3, space="PSUM") as ps1,
                tc.tile_pool(name="psv", bufs=2, space="PSUM") as psv,
                tc.tile_pool(name="psr", bufs=2, space="PSUM") as psr,
            ):
                rT_sb = rtp.tile([128, 128], F32R, tag="rT")
                nc.sync.dma_start(out=rT_sb[:], in_=rT[:])
                xT3 = xT.rearrange("(kt p) s -> p kt s", p=128)
                xbf3 = xbfT.rearrange("(kt p) s -> p kt s", p=128)

                def finish_rope(raw, ropes, h, cos_l, sin_l, cs_l):
                    rotps = psr.tile([128, CH1], F32, tag="rot", name="rotps")
                    nc.tensor.matmul(rotps[:], rT_sb[:], raw[:], start=True, stop=True)
                    t1 = worktp.tile([128, CH1], F32, tag="t1", name="t1")
                    nc.vector.tensor_mul(t1[:], rotps[:], sin_l[:])
                    t2 = worktp.tile([128, CH1], F32, tag="t2", name="t2")
                    nc.vector.tensor_mul(t2[:], raw[:].bitcast(F32), cos_l[:])
                    nc.vector.tensor_add(ropes[h][:, cs_l], t1[:], t2[:])

                for pas in range(2):
                    with tc.tile_pool(name=f"w1_{pas}", bufs=1) as wp1:
                        if pas == 0:
                            w_t = [wp1.tile([128, HG_], F32R, tag=f"wk{kt}", name=f"wk{kt}") for kt in range(KT)]
                            wT_dram, ropes = wkT, krope
                            wv_sb = wp1.tile([128, KT * HG_], BF16, tag="wv")
                        else:
                            w_t = [wp1.tile([128, HG_], F32R, tag=f"wq{kt}", name=f"wq{kt}") for kt in range(KT)]
                            wT_dram, ropes = wqT, qrope
                        for sc in range(NCH1):
                            cs = slice(sc * CH1, (sc + 1) * CH1)
                            cos_t = tabp.tile([128, CH1], F32, tag="cos")
                            nc.sync.dma_start(out=cos_t[:], in_=cosT[:, cs])
                            sin_t = tabp.tile([128, CH1], F32, tag="sin")
                            nc.sync.dma_start(out=sin_t[:], in_=sinT[:, cs])
                            if sc == 0:
                                for kt in range(KT):
                                    nc.sync.dma_start(
                                        out=w_t[kt][:], in_=wT_dram[kt * 128:(kt + 1) * 128, :]
                                    )
                            # x in 4 quarter-tiles (4 kt-groups each) so the
                            # first matmuls start after 1MB and the next
                            # chunk's stream overlaps this chunk's compute
                            xkq = []
                            for qq in range(4):
                                xq = xp1.tile([128, 4 * CH1], F32R, tag=f"xk{qq}", name=f"xk{qq}")
                                nc.sync.dma_start(
                                    out=xq[:].rearrange("p (kt s) -> p kt s", kt=4),
                                    in_=xT3[:, 4 * qq:4 * qq + 4, cs],
                                )
                                xkq.append(xq)
                            if pas == 0:
                                xv = xvp.tile([128, KT * CH1], BF16, tag="xv")
                                nc.sync.dma_start(
                                    out=xv[:].rearrange("p (kt s) -> p kt s", kt=KT),
                                    in_=xbf3[:, :, cs],
                                )
                            if pas == 0 and sc == 0:
                                nc.sync.dma_start(
                                    out=wv_sb[:].rearrange("p (kt j) -> p kt j", kt=KT),
                                    in_=wvT.rearrange("(kt p) j -> p kt j", p=128),
                                )
                            pending = None
                            for h in range(NH_):
                                ps = ps1.tile([128, CH1], F32, tag="qk")
                                for kt in range(KT):
                                    nc.tensor.matmul(
                                        ps[:],
                                        w_t[kt][:, h * 128:(h + 1) * 128],
                                        xkq[kt // 4][:, (kt % 4) * CH1:(kt % 4 + 1) * CH1],
                                        start=(kt == 0),
                                        stop=(kt == KT - 1),
                                    )
                                raw = workp.tile([128, CH1], F32R, tag="raw")
                                nc.vector.tensor_copy(raw[:], ps[:])
                                if pending is not None:
                                    finish_rope(*pending)
                                pending = (raw, ropes, h, cos_t, sin_t, cs)
                            finish_rope(*pending)

                            if pas == 0:
                                # v projection from a bf16 x stream (FWL-fast
                                # LDWEIGHTS), interleaved into the k pass
                                for t2i in range(CH1 // 128):
                                    t = sc * (CH1 // 128) + t2i
                                    vps = psv.tile([128, HG_], F32, tag="vps")
                                    for kt in range(KT):
                                        nc.tensor.matmul(
                                            vps[:],
                                            xv[:, kt * CH1 + t2i * 128: kt * CH1 + (t2i + 1) * 128],
                                            wv_sb[:, kt * HG_:(kt + 1) * HG_],
                                            start=(kt == 0),
                                            stop=(kt == KT - 1),
                                        )
                                    nc.scalar.copy(vslab[:, t * HG_:(t + 1) * HG_], vps[:])

            # ====== era 2: attention + output projection ======
            with (
                tc.tile_pool(name="w2", bufs=1) as wp2,
                tc.tile_pool(name="pslab", bufs=3) as pslabp,
                tc.tile_pool(name="ptpool", bufs=2) as ptp,
                tc.tile_pool(name="ctxpool", bufs=1) as ctxp,
                tc.tile_pool(name="ostage", bufs=2) as ostp,
                tc.tile_pool(name="psbig", bufs=4, space="PSUM") as psbig,
                tc.tile_pool(name="pssmall", bufs=1, space="PSUM") as pssmall,
                tc.tile_pool(name="psctx", bufs=2, space="PSUM") as psctx,
            ):
                ident_sb = wp2.tile([128, 128], BF16, tag="ident")
                nc.sync.dma_start(out=ident_sb[:], in_=ident[:])
                identf_sb = wp2.tile([128, 128], F32, tag="identf")
                nc.sync.dma_start(out=identf_sb[:], in_=identf[:])
                ones_sb = wp2.tile([1, 128], BF16, tag="onesr")
                nc.sync.dma_start(out=ones_sb[:], in_=onesr[:])
                mask_sb = wp2.tile([128, 128], F32, tag="mask")
                nc.sync.dma_start(out=mask_sb[:], in_=mask[:])
                wo_sb = wp2.tile([128, NH_ * H_], BF16, tag="wo")
                nc.sync.dma_start(
                    out=wo_sb[:].rearrange("p (j ho) -> p j ho", j=NH_),
                    in_=woT.rearrange("(j p) ho -> p j ho", p=128),
                )

                ctxT = [ctxp.tile([128, S_], BF16, tag=f"ctxT{h}", name=f"ctxT{h}") for h in range(NH_)]

                def emit_wo(st):
                    ostg = ostp.tile([128, H_], F32, tag="ostg", name="ostg")
                    for hoc in range(H_ // CH):
                        wops = psbig.tile([128, CH], F32, tag="sc", name="wops")
                        for j in range(NH_):
                            nc.tensor.matmul(
                                wops[:],
                                ctxT[j][:, st * 128:(st + 1) * 128],
                                wo_sb[:, j * H_ + hoc * CH: j * H_ + (hoc + 1) * CH],
                                start=(j == 0),
                                stop=(j == NH_ - 1),
                            )
                        if hoc % 2 == 0:
                            nc.scalar.copy(ostg[:, hoc * CH:(hoc + 1) * CH], wops[:])
                        else:
                            nc.vector.tensor_copy(ostg[:, hoc * CH:(hoc + 1) * CH], wops[:])
                    nc.sync.dma_start(out=out[st * 128:(st + 1) * 128, :], in_=ostg[:])

                for c in range(NCHUNK):
                    for h in range(NH_):
                        if c > 0:
                            emit_wo(4 * (c - 1) + h)

                        ptslab = ptp.tile([128, SQT * CH], BF16, tag="pt", name="pt")
                        pt3 = ptslab[:].rearrange("p (t q) -> p t q", t=SQT)

                        def do_transposes(pbf, sq):
                            # DMA xbar transpose: each 128-col block of the
                            # bf16 P slab lands transposed in the pt slab.
                            # All transposes stay on ONE engine: concurrent
                            # xbar transposes from two HWDGE engines race and
                            # corrupt data (measured 2e-2 rel err).
                            off = (sq - 4 * c) * 128
                            nc.sync.dma_start(
                                out=pt3[:, 0:sq + 1, off:off + 128],
                                in_=pbf[:, :(sq + 1) * 128],
                                transpose=True,
                            )

                        rcp4 = statp.tile([128, 4], F32, tag="rcp4")
                        pend_tr = None
                        for j, sq in enumerate(range(4 * c, 4 * c + 4)):
                            nch = c + 1
                            dw = (j + 1) * 128
                            scps_list = []
                            for kc in range(c):
                                scps = psbig.tile([128, CH], F32, tag="sc")
                                nc.tensor.matmul(
                                    scps[:],
                                    qrope[h][:, sq * 128:(sq + 1) * 128],
                                    krope[h][:, kc * CH:(kc + 1) * CH],
                                    start=True,
                                    stop=True,
                                )
                                scps_list.append((scps, CH))
                            dps = psbig.tile([128, CH], F32, tag="sc")
                            nc.tensor.matmul(
                                dps[:, :dw],
                                qrope[h][:, sq * 128:(sq + 1) * 128],
                                krope[h][:, c * CH: c * CH + dw],
                                start=True,
                                stop=True,
                            )
                            nc.vector.tensor_add(
                                dps[:, dw - 128:dw], dps[:, dw - 128:dw], mask_sb[:]
                            )
                            scps_list.append((dps, dw))

                            # row max (one PSUM operand per DVE op)
                            negm = statp.tile([128, 1], F32, tag="negm")
                            if nch == 1:
                                nc.vector.tensor_reduce(
                                    negm[:], dps[:, :dw], axis=AX.X, op=ALU.max, negate=True
                                )
                            else:
                                mx = statp.tile([128, 4], F32, tag="mx")
                                for kc, (scps, cols) in enumerate(scps_list):
                                    nc.vector.tensor_reduce(
                                        mx[:, kc:kc + 1], scps[:, :cols], axis=AX.X, op=ALU.max
                                    )
                                nc.vector.tensor_reduce(
                                    negm[:], mx[:, :nch], axis=AX.X, op=ALU.max, negate=True
                                )

                            # unnormalized P in bf16; row sums accumulate on ACT
                            pbf = pslabp.tile([128, S_], BF16, tag="pbf", name="pbf")
                            ssum = statp.tile([128, 4], F32, tag="ssum")
                            for kc, (scps, cols) in enumerate(scps_list):
                                nc.scalar.activation(
                                    pbf[:, kc * CH: kc * CH + cols],
                                    scps[:, :cols],
                                    ACTF.Exp,
                                    bias=negm[:],
                                    accum_out=ssum[:, kc:kc + 1],
                                )
                            if nch == 1:
                                nc.vector.reciprocal(rcp4[:, j:j + 1], ssum[:, 0:1])
                            else:
                                rsum = statp.tile([128, 1], F32, tag="rsum")
                                nc.vector.tensor_reduce(
                                    rsum[:], ssum[:, :nch], axis=AX.X, op=ALU.add
                                )
                                nc.vector.reciprocal(rcp4[:, j:j + 1], rsum[:])

                            if pend_tr is not None:
                                do_transposes(*pend_tr)
                            pend_tr = (pbf, sq)
                        do_transposes(*pend_tr)

                        tmax = 4 * c + 4
                        ctxps = psctx.tile([128, CH], F32, tag="ctx")
                        for t in range(tmax):
                            c0 = max(0, (t - 4 * c) * 128)
                            nc.tensor.matmul(
                                ctxps[:, c0:CH],
                                vslab[:, t * HG_ + h * 128: t * HG_ + (h + 1) * 128],
                                ptslab[:, t * CH + c0: t * CH + CH],
                                start=(t == 0),
                                stop=(t == tmax - 1),
                            )
                        # broadcast 1/rowsum along partitions and normalize ctx
                        rowps = pssmall.tile([1, CH], F32, tag="rcpT")
                        for j in range(4):
                            nc.tensor.transpose(
                                rowps[0:1, j * 128:(j + 1) * 128],
                                rcp4[:, j:j + 1],
                                identf_sb[:],
                            )
                        rrow = statp.tile([1, CH], BF16, tag="rrow")
                        nc.vector.tensor_copy(rrow[:], rowps[:])
                        bcps = pssmall.tile([128, CH], F32, tag="bcps")
                        nc.tensor.matmul(bcps[:], ones_sb[:], rrow[:], start=True, stop=True)
                        bcsb = statp.tile([128, CH], BF16, tag="bcsb")
                        nc.vector.tensor_copy(bcsb[:], bcps[:])
                        nc.vector.tensor_mul(
                            ctxT[h][:, c * CH:(c + 1) * CH], ctxps[:], bcsb[:]
                        )

                for h in range(NH_):
                    emit_wo(4 * (NCHUNK - 1) + h)

    nc.compile()
    return nc


def _make_tables(S_, D_=128):
    inv_freq = 1.0 / (ROPE_BASE ** (np.arange(0, D_, 2, dtype=np.float32) / D_))
    pos = np.arange(S_, dtype=np.float32)
    ang = pos[:, None] * inv_freq[None, :]
    ang = np.concatenate([ang, ang], axis=1)
    return (
        np.cos(ang).T.astype(np.float32).copy(),
        np.sin(ang).T.astype(np.float32).copy(),
    )


def _make_rot_T(D_=128):
    R = np.zeros((D_, D_), dtype=np.float32)
    half = D_ // 2
    for d in range(half):
        R[d, d + half] = -1.0
    for d in range(half, D_):
        R[d, d - half] = 1.0
    return R.T.copy()


def _make_mask(mask_val=-1e30):
    m = np.zeros((128, 128), dtype=np.float32)
    m[np.triu_indices(128, k=1)] = mask_val
    return m


def kernel(x, Wq, Wk, Wv, Wo):
    """Full inputs in, full output out. Shards over 8 NeuronCores internally."""
    global LAST_RESULTS
    x = np.ascontiguousarray(np.asarray(x, dtype=np.float32))
    Wq = np.asarray(Wq, dtype=np.float32)
    Wk = np.asarray(Wk, dtype=np.float32)
    Wv = np.asarray(Wv, dtype=np.float32)
    Wo = np.asarray(Wo, dtype=np.float32)

    if "nc" not in _NC_CACHE:
        _NC_CACHE["nc"] = _build()
    nc = _NC_CACHE["nc"]

    scale = np.sqrt(np.float32(D))
    cosT, sinT = _make_tables(S)
    rT = _make_rot_T()
    identb = np.eye(128, dtype=ml_dtypes.bfloat16)
    identf = np.eye(128, dtype=np.float32)
    onesr = np.ones((1, 128), dtype=ml_dtypes.bfloat16)
    maskt = _make_mask()

    WqT = Wq.T * scale                    # [H, 16*D], scale folded into q path
    WkT = np.ascontiguousarray(Wk.T)
    WvT_bf = Wv.T.astype(ml_dtypes.bfloat16)
    WoT_bf = Wo.T.astype(ml_dtypes.bfloat16)   # [H(in=ctx), H(out)] rows = ctx hidden

    in_maps = []
    for c in range(N_CORES):
        b, g = divmod(c, NH)
        js = slice(g * HG, (g + 1) * HG)
        xT_b = np.ascontiguousarray(x[b].T)
        in_maps.append({
            "xT": xT_b,
            "xbfT": xT_b.astype(ml_dtypes.bfloat16),
            "wqT": np.ascontiguousarray(WqT[:, js]).astype(np.float32),
            "wkT": np.ascontiguousarray(WkT[:, js]),
            "wvT": np.ascontiguousarray(WvT_bf[:, js]),
            "woT": np.ascontiguousarray(WoT_bf[js, :]),
            "cosT": cosT,
            "sinT": sinT,
            "rT": rT,
            "ident": identb,
            "identf": identf,
            "onesr": onesr,
            "mask": maskt,
        })

    LAST_RESULTS = run_bass_kernel_spmd(
        nc, in_maps, core_ids=list(range(N_CORES)), trace=TRACE
    )
    res = LAST_RESULTS.results

    out = np.zeros((B, S, H), dtype=np.float32)
    for c in range(N_CORES):
        b = c // NH
        out[b] += res[c]["out"]
    return out
